# revision 1
# baseline (speedup 1.0000x reference)
"""Trainium2 Bass kernel for nn_CausalAttention_76304388981436.

Full-input contract: kernel(**inputs) -> [2, 2048, 512] f32.

Sharding (8 cores, single SPMD program): core c = (batch b=c//4, head-pair
hp=c%4).  Each core computes attention for its 2 heads over the full 2048
sequence of its batch, producing a partial output  attnT_2h @ Wo[2h-rows]
[2048, 512]; host sums the 4 head-pair partials per batch and adds bo.

Device-side math per core (all matmuls fp32r, transposed-attention layout):
  qT[128i, 2048n] = wq2^T x^T   (wq2 pre-scaled by 1/8 on host)
  kT[128i, 2048n] = wk2^T x^T
  v [2048n, 2x65] = x wv2       (col 64 of each head block memset to 1.0 ->
                                 PV matmul also produces softmax denominator)
  P'[2048q, 1032] = qT_h^T rel_embT_rev   (rel table pre-reversed on host)
  PR dram [2048q, 2048c] fp16: P' chunks + clamp-value pads, laid out so the
      relative-position skew  bias[j,q] = P[q, clip(q-j,-512,512)+512]
      becomes a plain strided read:  flat = 2047*fq + pj + (1535 - A).
  S^T tile [128j, 512q] = kT_h^T qT_h (+ clamp-bias matmul | + skew-tile add)
  expS = exp(S^T)   (no max-subtraction: |logits| <~ 10, fp32-safe)
  outT_h [65, 512q] += v_ext_h^T expS   (row 64 = denominator)
  attnT_h = outT_h[0:64] * (1/denom broadcast)
  partial[2048q, 512] = attnT^T wo2
"""
import numpy as np
import ml_dtypes

import concourse.bass as bass
import concourse.mybir as mybir
import concourse.tile as tile
from concourse.bass_utils import run_bass_kernel_spmd

F32 = mybir.dt.float32
F32R = mybir.dt.float32r
F16 = mybir.dt.float16
BF16 = mybir.dt.bfloat16
AF = mybir.ActivationFunctionType

N = 2048          # sequence length
D = 512           # model dim
HD = 64           # head dim
NQC = 4           # q-chunks of 512
NJT = 16          # j-tiles of 128
W = 2048          # padded PR row width


def _split_multiwaits(nc):
    """This walrus build rejects >1 sync wait per instruction; split extra
    waits onto single-wait NoOps on the same engine just before it."""
    for func in nc.m.functions:
        for block in func.blocks:
            new_instrs = []
            for inst in block.instructions:
                si = inst.sync_info
                if si is not None and si.on_wait and len(si.on_wait) > 1:
                    waits = list(si.on_wait)
                    for w in waits[:-1]:
                        new_instrs.append(mybir.InstNoOp(
                            name=nc.get_next_instruction_name(),
                            engine=inst.engine,
                            bass_nofuse=True,
                            sync_info=mybir.SyncInfo(on_wait=[w], on_update=[]),
                        ))
                    si.on_wait = waits[-1:]
                new_instrs.append(inst)
            block.instructions = new_instrs


def _r(ap):
    return ap.bitcast(F32R)


def build_kernel():
    nc = bass.Bass("TRN2")
    xT = nc.dram_tensor("xT", [D, N], F32, kind="ExternalInput")
    wq2 = nc.dram_tensor("wq2", [D, 128], F32, kind="ExternalInput")
    wk2 = nc.dram_tensor("wk2", [D, 128], F32, kind="ExternalInput")
    wv2 = nc.dram_tensor("wv2", [D, 128], F32, kind="ExternalInput")
    wo2 = nc.dram_tensor("wo2", [128, D], F32, kind="ExternalInput")
    relT = nc.dram_tensor("relT", [128, W], BF16, kind="ExternalInput")
    relbc = nc.dram_tensor("relbc", [128, 256], BF16, kind="ExternalInput")
    ones2 = nc.dram_tensor("ones2", [128, 2, 1], F32, kind="ExternalInput")
    out = nc.dram_tensor("out", [N, D], F32, kind="ExternalOutput")

    with tile.TileContext(nc) as tc:
        _build_body(nc, tc, xT, wq2, wk2, wv2, wo2, relT, relbc, ones2, out)
    _split_multiwaits(nc)
    return nc


def _build_body(nc, tc, xT, wq2, wk2, wv2, wo2, relT, relbc, ones2, out):
    from contextlib import ExitStack
    ctx = ExitStack()
    consts = ctx.enter_context(tc.tile_pool(name="consts", bufs=1))
    qkv = ctx.enter_context(tc.tile_pool(name="qkv", bufs=1))
    pcast = ctx.enter_context(tc.tile_pool(name="pcast", bufs=6))
    skew = ctx.enter_context(tc.tile_pool(name="skew", bufs=6))
    exps = ctx.enter_context(tc.tile_pool(name="exps", bufs=18))
    small = ctx.enter_context(tc.tile_pool(name="small", bufs=4))
    outc = ctx.enter_context(tc.tile_pool(name="outc", bufs=2))
    psa = ctx.enter_context(tc.tile_pool(name="psa", bufs=4, space="PSUM"))
    pso = ctx.enter_context(tc.tile_pool(name="pso", bufs=2, space="PSUM"))
    pdram = ctx.enter_context(tc.tile_pool(name="pdram", bufs=1, space="DRAM"))

    # ---- load constants / inputs ----
    sxT = [consts.tile([128, N], F32, name=f"xT{i}", tag=f"xT{i}") for i in range(4)]
    for i in range(4):
        nc.sync.dma_start(out=_r(sxT[i][:]), in_=_r(xT[i * 128:(i + 1) * 128, :]))
    swq = [consts.tile([128, 128], F32, name=f"wq{i}", tag=f"wq{i}") for i in range(4)]
    swk = [consts.tile([128, 128], F32, name=f"wk{i}", tag=f"wk{i}") for i in range(4)]
    swv = [consts.tile([128, 128], F32, name=f"wv{i}", tag=f"wv{i}") for i in range(4)]
    for i in range(4):
        nc.sync.dma_start(out=_r(swq[i][:]), in_=_r(wq2[i * 128:(i + 1) * 128, :]))
        nc.sync.dma_start(out=_r(swk[i][:]), in_=_r(wk2[i * 128:(i + 1) * 128, :]))
        nc.sync.dma_start(out=_r(swv[i][:]), in_=_r(wv2[i * 128:(i + 1) * 128, :]))
    swo = consts.tile([128, D], F32, name="wo", tag="wo")
    nc.sync.dma_start(out=_r(swo[:]), in_=_r(wo2[:, :]))
    srelT = consts.tile([128, W], BF16, name="relT", tag="relT")
    nc.sync.dma_start(out=srelT[:], in_=relT[:, :])
    srelbc = consts.tile([128, 256], BF16, name="relbc", tag="relbc")
    nc.sync.dma_start(out=srelbc[:], in_=relbc[:, :])
    sones = consts.tile([128, 2, 1], F32, name="ones2", tag="ones2")
    nc.sync.dma_start(out=_r(sones[:]), in_=_r(ones2[:, :, :]))

    # ---- projections ----
    qT = qkv.tile([128, N], BF16, name="qT", tag="qT")
    kT = qkv.tile([128, N], BF16, name="kT", tag="kT")
    for nchunk in range(NQC):
        ns = slice(nchunk * 512, nchunk * 512 + 512)
        for dst, w in ((qT, swq), (kT, swk)):
            ps = psa.tile([128, 512], F32, name="ps", tag="ps")
            for c in range(4):
                nc.tensor.matmul(ps[:], _r(w[c][:]), _r(sxT[c][:, ns]),
                                 start=(c == 0), stop=(c == 3))
            nc.vector.tensor_copy(out=dst[:, ns], in_=ps[:])
    # v in natural layout with ones column per head block
    vt = [qkv.tile([128, 2, 65], BF16, name=f"v{t}", tag=f"v{t}") for t in range(NJT)]
    for t in range(NJT):
        nst = slice(t * 128, t * 128 + 128)
        ps = psa.tile([128, 128], F32, name="ps", tag="ps")
        for c in range(4):
            nc.tensor.matmul(ps[:], _r(sxT[c][:, nst]), _r(swv[c][:]),
                             start=(c == 0), stop=(c == 3))
        nc.vector.tensor_copy(out=vt[t][:, :, 64:65], in_=sones[:])
        nc.vector.tensor_copy(out=vt[t][:, :, 0:64],
                              in_=ps[:].rearrange("p (h d) -> p h d", h=2))

    # ---- P' phase: PR[q, col] = q_h . rel_ext[col]  -> PR dram (fp16) ----
    # rel_ext (host) already encodes reversal + clamp padding per column.
    prd = [pdram.tile([N, W], F16, name=f"pr{h}", tag=f"pr{h}") for h in range(2)]
    def emit_P():
        # both heads' K=64 matmuls adjacent: disjoint PE row-groups (base
        # partition 0 / 64) execute concurrently on the tiled array
        for qt in range(NJT):
            qs = slice(qt * 128, qt * 128 + 128)
            rows = slice(qt * 128, qt * 128 + 128)
            for ci in range(4):
                cs = slice(ci * 512, ci * 512 + 512)
                pss = []
                for h in range(2):
                    hs = slice(h * 64, h * 64 + 64)
                    ps = psa.tile([128, 512], F32, name="ps", tag="ps")
                    nc.tensor.matmul(ps[:], qT[hs, qs], srelT[hs, cs],
                                     start=True, stop=True,
                                     tile_position=(h * 64, 0))
                    pss.append(ps)
                for h in range(2):
                    ct = pcast.tile([128, 512], F16, name="pc", tag="pc")
                    nc.vector.tensor_copy(out=ct[:], in_=pss[h][:])
                    nc.gpsimd.dma_start(out=prd[h][rows, cs], in_=ct[:])

    # ---- attention ----
    attnT = qkv.tile([128, N], F32, name="attnT", tag="attnT")
    rdd = [pdram.tile([1, 512], F32, name=f"rdd{i}", tag=f"rdd{i}") for i in range(8)]
    def emit_attn():
        for qc in range(NQC):
            qs = slice(qc * 512, qc * 512 + 512)
            pos = [pso.tile([65, 512], F32, name="po", tag=f"po{h}")
                   for h in range(2)]
            ets = {0: [], 1: []}
            for jt in range(NJT):
                js = slice(jt * 128, jt * 128 + 128)
                A = qc * 512 + 512 - 128 * jt
                pss = []
                for h in range(2):
                    hs = slice(h * 64, h * 64 + 64)
                    ps = psa.tile([128, 512], F32, name="ps", tag="ps")
                    if A <= -512 or A >= 1152:
                        bc = 0 if A <= -512 else 128
                        nc.tensor.matmul(ps[:], kT[hs, js], qT[hs, qs],
                                         start=True, stop=False,
                                         tile_position=(h * 64, 0))
                        nc.tensor.matmul(ps[:], srelbc[hs, bc:bc + 128],
                                         qT[hs, qs], start=False, stop=True,
                                         tile_position=(h * 64, 0))
                    else:
                        nc.tensor.matmul(ps[:], kT[hs, js], qT[hs, qs],
                                         start=True, stop=True,
                                         tile_position=(h * 64, 0))
                    pss.append(ps)
                for h in range(2):
                    A2 = A
                    if not (A2 <= -512 or A2 >= 1152):
                        bt = skew.tile([128, 512], F16, name="skew", tag="skew")
                        srcap = bass.AP(tensor=prd[h].tensor,
                                        offset=prd[h].offset + qc * 512 * W + (1535 - A2),
                                        ap=[[2047, 512], [1, 128]])
                        nc.scalar.dma_start(out=bt[:], in_=srcap, transpose=True)
                        nc.vector.tensor_add(out=pss[h][:], in0=pss[h][:], in1=bt[:])
                    et = exps.tile([128, 512], BF16, name="expS", tag="expS")
                    nc.scalar.activation(out=et[:], in_=pss[h][:], func=AF.Exp)
                    ets[h].append(et)
            for jt in range(NJT):
                for h in range(2):
                    nc.tensor.matmul(pos[h][:], vt[jt][:, h, :], ets[h][jt][:],
                                     start=(jt == 0), stop=(jt == NJT - 1))
            for h in range(2):
                hs = slice(h * 64, h * 64 + 64)
                po = pos[h]
                rd = small.tile([1, 512], F32, name="rd", tag="rd")
                nc.vector.reciprocal(out=rd[:], in_=po[64:65, :])
                slot = h * 4 + qc
                nc.sync.dma_start(out=rdd[slot][:], in_=rd[:])
                rdb = small.tile([64, 512], F32, name="rdb", tag="rdb")
                bcast = bass.AP(tensor=rdd[slot].tensor, offset=rdd[slot].offset,
                                ap=[[0, 64], [1, 512]])
                nc.sync.dma_start(out=rdb[:], in_=bcast)
                nc.vector.tensor_mul(out=_r(attnT[hs, qs]), in0=po[0:64, :],
                                     in1=rdb[:])

    emit_P()
    emit_attn()

    # ---- output projection (partial over this core's 2 heads) ----
    for qt in range(NJT):
        qs = slice(qt * 128, qt * 128 + 128)
        ps = psa.tile([128, 512], F32, name="ps", tag="ps")
        nc.tensor.matmul(ps[:], _r(attnT[:, qs]), _r(swo[:]),
                         start=True, stop=True)
        ot = outc.tile([128, 512], F32, name="oc", tag="oc")
        nc.vector.tensor_copy(out=ot[:], in_=ps[:])
        nc.gpsimd.dma_start(out=out[qs, :], in_=ot[:])
    ctx.close()


_NC_CACHE = [None]


def _get_nc():
    if _NC_CACHE[0] is None:
        _NC_CACHE[0] = build_kernel()
    return _NC_CACHE[0]


def make_in_maps(x, Wq, Wkv, Wo, bo, rel_emb):
    xT = [np.ascontiguousarray(x[b].T).astype(np.float32) for b in range(2)]
    cols = np.arange(W)
    idx = np.clip(1535 - cols, 0, 1024)
    relT = np.empty((128, W), np.float32)
    relT[0:64] = rel_emb[idx].T
    relT[64:128] = relT[0:64]
    relT = relT.astype(ml_dtypes.bfloat16)          # reversed rel table
    relbc = np.empty((128, 256), np.float32)
    relbc[0:64, 0:128] = rel_emb[0][:, None]       # clamp-low value
    relbc[0:64, 128:256] = rel_emb[1024][:, None]  # clamp-high value
    relbc[64:128] = relbc[0:64]
    relbc = relbc.astype(ml_dtypes.bfloat16)
    in_maps = []
    for c in range(8):
        b, hp = c // 4, c % 4
        cs = slice(hp * 128, hp * 128 + 128)
        in_maps.append({
            "xT": xT[b],
            "wq2": np.ascontiguousarray(Wq[:, cs] / 8.0).astype(np.float32),
            "wk2": np.ascontiguousarray(Wkv[:, :512][:, cs]).astype(np.float32),
            "wv2": np.ascontiguousarray(Wkv[:, 512:][:, cs]).astype(np.float32),
            "wo2": np.ascontiguousarray(Wo[cs, :]).astype(np.float32),
            "relT": relT,
            "relbc": relbc,
            "ones2": np.ones((128, 2, 1), np.float32),
        })
    return in_maps


def run(x, Wq, Wkv, Wo, bo, rel_emb, trace=False, trace_cores=None):
    nc = _get_nc()
    in_maps = make_in_maps(x, Wq, Wkv, Wo, bo, rel_emb)
    res = run_bass_kernel_spmd(nc, in_maps, core_ids=list(range(8)),
                               trace=trace, trace_cores=trace_cores)
    out = np.zeros((2, N, D), np.float32)
    for c in range(8):
        out[c // 4] += res.results[c]["out"]
    out += np.asarray(bo, np.float32)[None, None, :]
    return out, res


def kernel(x, Wq, Wkv, Wo, bo, rel_emb):
    out, _ = run(np.asarray(x), np.asarray(Wq), np.asarray(Wkv),
                 np.asarray(Wo), np.asarray(bo), np.asarray(rel_emb))
    return out



# revision 9
# speedup vs baseline: 1.5408x; 1.5408x over previous
"""Trainium2 Bass kernel for nn_CausalAttention_76304388981436 (v2).

Full-input contract: kernel(**inputs) -> [2, 2048, 512] f32.

Sharding (8 cores, single SPMD program): core c = (batch b=c//4, head-pair
hp=c%4).  Each core computes attention for its 2 heads over the full 2048
sequence of its batch and ships per-head UNNORMALIZED outputs
  p_h[2048 q, 512] = (exp-weighted attnT_h)^T wo[h rows]      (bf16)
plus per-head softmax denominators den[2, 2048] (f32); the host divides,
sums the 4 head-pair partials per batch and adds bo.

Device-side per core (transposed-attention layout, S^T tiles [j, q]):
  qT/kT[128 hd, 2048 n] = w^T x^T  (fp32r matmuls; wq pre-scaled by 1/8)
  kTlo/kThi = kT + rel_clamp_col   (folds fully-clamped rel bias into K)
  v[2048 j, 2, 65] with ones col 64 -> PV also yields denominators
  P' table PR[h][qc] dram [512 q, 2048 c] fp16 = qT_h^T rel_rev (band only)
  skew bias tiles come back via ONE 3D-output transposed DMA per (h, qc):
     src [[2047, 512],[1, 128*n]] -> out[128 j, n(jt), 512 q]
  S^T psum pair [128 j, 1024 (h0|h1)]: kq matmul + identity-matmul skew add
  expS = exp(S^T) (single ACT op per pair; no max-subtract: |logits| <~ 12)
  pos[65, 1024] += v_h^T expS_h ; row 64 = denominator
  out pair psum [128 q, 1024 (h0|h1)] = attnTu_h^T wo_h
P'(qc+1) and outproj(qc-1) are interleaved into attn(qc)'s jt loop so all
engines stay busy and the PE never idles long enough to re-throttle.
"""
import numpy as np
import ml_dtypes

import concourse.bass as bass
import concourse.mybir as mybir
import concourse.tile as tile
from concourse.bass_utils import run_bass_kernel_spmd

F32 = mybir.dt.float32
F32R = mybir.dt.float32r
F16 = mybir.dt.float16
BF16 = mybir.dt.bfloat16
AF = mybir.ActivationFunctionType

N = 2048          # sequence length
D = 512           # model dim
W = 2048          # P' table row width
NJT = 16          # j-tiles of 128
NQC = 4           # q-chunks of 512

# interior jt range [lo, hi) per qc  (A = 512*qc + 512 - 128*jt in (-512, 1152))
INT_RANGE = {0: (0, 8), 1: (0, 12), 2: (4, 16), 3: (8, 16)}
# fully-clamped tiles: low (A <= -512) uses rel[0], high (A >= 1152) uses rel[1024]
EXT_LO = {0: range(8, 16), 1: range(12, 16), 2: range(0, 0), 3: range(0, 0)}
EXT_HI = {0: range(0, 0), 1: range(0, 0), 2: range(0, 4), 3: range(0, 8)}
# src offset of the merged skew read inside PR block qc: 1535 - A(qc, jt_lo)
OFF0 = {0: 1023, 1: 511, 2: 511, 3: 511}
# alive c-chunk span [ci_lo, ci_hi) per (qc, sub) for P' compute/write
ALIVE = {0: (1, 4), 1: (0, 4), 2: (0, 4), 3: (0, 3)}


def _split_multiwaits(nc):
    """This walrus build rejects >1 sync wait per instruction; split extra
    waits onto single-wait NoOps on the same engine just before it."""
    for func in nc.m.functions:
        for block in func.blocks:
            new_instrs = []
            for inst in block.instructions:
                si = inst.sync_info
                if si is not None and si.on_wait and len(si.on_wait) > 1:
                    waits = list(si.on_wait)
                    for w in waits[:-1]:
                        new_instrs.append(mybir.InstNoOp(
                            name=nc.get_next_instruction_name(),
                            engine=inst.engine,
                            bass_nofuse=True,
                            sync_info=mybir.SyncInfo(on_wait=[w], on_update=[]),
                        ))
                    si.on_wait = waits[-1:]
                new_instrs.append(inst)
            block.instructions = new_instrs


def _r(ap):
    return ap.bitcast(F32R)


def build_kernel():
    nc = bass.Bass("TRN2")
    xT = nc.dram_tensor("xT", [D, N], F32, kind="ExternalInput")
    wq2 = nc.dram_tensor("wq2", [D, 128], F32, kind="ExternalInput")
    wk2 = nc.dram_tensor("wk2", [D, 128], F32, kind="ExternalInput")
    wv2 = nc.dram_tensor("wv2", [D, 128], F32, kind="ExternalInput")
    wo2 = nc.dram_tensor("wo2", [128, D], BF16, kind="ExternalInput")
    relT = nc.dram_tensor("relT", [128, W], BF16, kind="ExternalInput")
    clampc = nc.dram_tensor("clampc", [128, 2], F32, kind="ExternalInput")
    ident = nc.dram_tensor("ident", [128, 128], F16, kind="ExternalInput")
    out0 = nc.dram_tensor("out0", [N, D], BF16, kind="ExternalOutput")
    out1 = nc.dram_tensor("out1", [N, D], BF16, kind="ExternalOutput")
    den = nc.dram_tensor("den", [2, N], F32, kind="ExternalOutput")

    with tile.TileContext(nc) as tc:
        _build_body(nc, tc, xT, wq2, wk2, wv2, wo2, relT, clampc, ident,
                    out0, out1, den)
    _split_multiwaits(nc)
    return nc


def _build_body(nc, tc, xT, wq2, wk2, wv2, wo2, relT, clampc, ident,
                out0, out1, den):
    from contextlib import ExitStack
    ctx = ExitStack()
    consts = ctx.enter_context(tc.tile_pool(name="consts", bufs=1))
    qkv = ctx.enter_context(tc.tile_pool(name="qkv", bufs=1))
    stgp = ctx.enter_context(tc.tile_pool(name="stgp", bufs=4))
    skwp = ctx.enter_context(tc.tile_pool(name="skwp", bufs=2))
    etp = ctx.enter_context(tc.tile_pool(name="etp", bufs=6))
    ostg = ctx.enter_context(tc.tile_pool(name="ostg", bufs=2))
    small = ctx.enter_context(tc.tile_pool(name="small", bufs=4))
    psS = ctx.enter_context(tc.tile_pool(name="psS", bufs=2, space="PSUM"))
    psPos = ctx.enter_context(tc.tile_pool(name="psPos", bufs=1, space="PSUM"))
    psScr = ctx.enter_context(tc.tile_pool(name="psScr", bufs=1, space="PSUM"))
    pdram = ctx.enter_context(tc.tile_pool(name="pdram", bufs=1, space="DRAM"))

    # ---- load constants / inputs (sync queue) ----
    sxT = [consts.tile([128, N], F32, name=f"xT{i}", tag=f"xT{i}") for i in range(4)]
    for i in range(4):
        nc.sync.dma_start(out=_r(sxT[i][:]), in_=_r(xT[i * 128:(i + 1) * 128, :]))
    swq = [consts.tile([128, 128], F32, name=f"wq{i}", tag=f"wq{i}") for i in range(4)]
    swk = [consts.tile([128, 128], F32, name=f"wk{i}", tag=f"wk{i}") for i in range(4)]
    swv = [consts.tile([128, 128], F32, name=f"wv{i}", tag=f"wv{i}") for i in range(4)]
    for i in range(4):
        nc.sync.dma_start(out=_r(swq[i][:]), in_=_r(wq2[i * 128:(i + 1) * 128, :]))
        nc.sync.dma_start(out=_r(swk[i][:]), in_=_r(wk2[i * 128:(i + 1) * 128, :]))
        nc.sync.dma_start(out=_r(swv[i][:]), in_=_r(wv2[i * 128:(i + 1) * 128, :]))
    swo = consts.tile([128, D], BF16, name="wo", tag="wo")
    nc.sync.dma_start(out=swo[:], in_=wo2[:, :])
    srelT = consts.tile([128, W], BF16, name="relT", tag="relT")
    nc.sync.dma_start(out=srelT[:], in_=relT[:, :])
    sclamp = consts.tile([128, 2], F32, name="clampc", tag="clampc")
    nc.sync.dma_start(out=_r(sclamp[:]), in_=_r(clampc[:, :]))
    sident = consts.tile([128, 128], F16, name="ident", tag="ident")
    nc.sync.dma_start(out=sident[:], in_=ident[:, :])

    # ---- q/k projections (copies on ACT; PE fp32r) ----
    qT = qkv.tile([128, N], BF16, name="qT", tag="qT")
    kT = qkv.tile([128, N], BF16, name="kT", tag="kT")
    kTlo = qkv.tile([128, N], BF16, name="kTlo", tag="kTlo")
    kThi = qkv.tile([128, N], BF16, name="kThi", tag="kThi")
    for nchunk in range(NQC):
        ns = slice(nchunk * 512, nchunk * 512 + 512)
        ps = psS.tile([128, 1024], F32, name="psqk", tag="ps")
        for c in range(4):
            nc.tensor.matmul(ps[:, 0:512], _r(swq[c][:]), _r(sxT[c][:, ns]),
                             start=(c == 0), stop=(c == 3))
        for c in range(4):
            nc.tensor.matmul(ps[:, 512:1024], _r(swk[c][:]), _r(sxT[c][:, ns]),
                             start=(c == 0), stop=(c == 3))
        nc.scalar.activation(out=qT[:, ns], in_=ps[:, 0:512], func=AF.Copy)
        nc.scalar.activation(out=kT[:, ns], in_=ps[:, 512:1024], func=AF.Copy)
    # clamp-folded K variants (per-partition scalar add)
    nc.vector.tensor_scalar_add(out=kTlo[:], in0=kT[:], scalar1=sclamp[:, 0:1])
    nc.vector.tensor_scalar_add(out=kThi[:], in0=kT[:], scalar1=sclamp[:, 1:2])

    # ---- P' table blocks in DRAM ----
    prd = {}
    for h in range(2):
        for qc in range(NQC):
            prd[(h, qc)] = pdram.tile([512, W], F16, name=f"pr{h}_{qc}",
                                      tag=f"pr{h}_{qc}")

    def emit_P_pair(qc, sub, ci, stg):
        """One cross-head P' matmul pair [128 q, 512 c] + casts into staging."""
        qs = slice(qc * 512 + sub * 128, qc * 512 + sub * 128 + 128)
        cs = slice(ci * 512, ci * 512 + 512)
        ps = psScr.tile([128, 1024], F32, name="psP", tag="scr")
        for h in range(2):
            hs = slice(h * 64, h * 64 + 64)
            nc.tensor.matmul(ps[:, h * 512:h * 512 + 512], qT[hs, qs],
                             srelT[hs, cs], start=True, stop=True,
                             tile_position=(h * 64, 0))
        for h in range(2):
            nc.vector.tensor_copy(out=stg[h][:, cs], in_=ps[:, h * 512:h * 512 + 512])

    def emit_P_write(qc, sub, stg):
        ci_lo, ci_hi = ALIVE[qc]
        cs = slice(ci_lo * 512, ci_hi * 512)
        rows = slice(sub * 128, sub * 128 + 128)
        for h in range(2):
            nc.gpsimd.dma_start(out=prd[(h, qc)][rows, cs], in_=stg[h][:, cs])

    def emit_P_block(qc):
        """Generator yielding emission closures for P'(qc) (pairs+writes)."""
        ci_lo, ci_hi = ALIVE[qc]
        for sub in range(4):
            stg = [stgp.tile([128, W], F16, name=f"stg{h}", tag=f"stg{h}")
                   for h in range(2)]
            for ci in range(ci_lo, ci_hi):
                yield lambda qc=qc, sub=sub, ci=ci, stg=stg: emit_P_pair(qc, sub, ci, stg)
            yield lambda qc=qc, sub=sub, stg=stg: emit_P_write(qc, sub, stg)

    def emit_skew_read(qc):
        """One 3D-output transposed DMA per head covering all interior jt."""
        jt_lo, jt_hi = INT_RANGE[qc]
        n = jt_hi - jt_lo
        tiles = []
        for h in range(2):
            t = skwp.tile([128, n, 512], F16, name=f"skw{h}", tag=f"skw{h}")
            src = bass.AP(tensor=prd[(h, qc)].tensor,
                          offset=prd[(h, qc)].offset + OFF0[qc],
                          ap=[[2047, 512], [1, 128 * n]])
            nc.sync.dma_start(out=t[:], in_=src, transpose=True)
            tiles.append(t)
        return tiles

    # ---- v projection (natural layout, ones col for denominators) ----
    vt = [qkv.tile([128, 2, 65], BF16, name=f"v{t}", tag=f"v{t}") for t in range(NJT)]
    def emit_v():
        for t in range(NJT):
            nst = slice(t * 128, t * 128 + 128)
            ps = psS.tile([128, 128], F32, name="psv", tag="ps")
            for c in range(4):
                nc.tensor.matmul(ps[:], _r(sxT[c][:, nst]), _r(swv[c][:]),
                                 start=(c == 0), stop=(c == 3))
            nc.vector.memset(vt[t][:, :, 64:65], 1.0)
            nc.vector.tensor_copy(out=vt[t][:, :, 0:64],
                                  in_=ps[:].rearrange("p (h d) -> p h d", h=2))

    # ---- attention ----
    attnTu = qkv.tile([128, N], BF16, name="attnTu", tag="attnTu")

    def emit_outproj_chunk(qc, qt):
        """Out-proj for one 128-q block: cross-head pair + copy + 2 writes."""
        qs = slice(qc * 512 + qt * 128, qc * 512 + qt * 128 + 128)
        ps = psScr.tile([128, 1024], F32, name="psO", tag="scr")
        for h in range(2):
            hs = slice(h * 64, h * 64 + 64)
            nc.tensor.matmul(ps[:, h * 512:h * 512 + 512], attnTu[hs, qs],
                             swo[hs, :], start=True, stop=True,
                             tile_position=(h * 64, 0))
        ot = ostg.tile([128, 1024], BF16, name="oc", tag="oc")
        if qt % 2 == 0:
            nc.scalar.activation(out=ot[:], in_=ps[:], func=AF.Copy)
        else:
            nc.vector.tensor_copy(out=ot[:], in_=ps[:])
        nc.gpsimd.dma_start(out=out0[qs, :], in_=ot[:, 0:512])
        nc.gpsimd.dma_start(out=out1[qs, :], in_=ot[:, 512:1024])

    PV_DELAY = 3

    def emit_attn_qc(qc, skw, fillers):
        jt_lo, jt_hi = INT_RANGE[qc]
        qs = slice(qc * 512, qc * 512 + 512)
        pos = psPos.tile([65, 1024], F32, name="pos", tag="pos")
        ets = [None] * NJT

        def emit_pv(jt):
            for h in range(2):
                nc.tensor.matmul(pos[:, h * 512:h * 512 + 512], vt[jt][:, h, :],
                                 ets[jt][:, h * 512:h * 512 + 512],
                                 start=(jt == 0), stop=(jt == NJT - 1))

        for jt in range(NJT):
            js = slice(jt * 128, jt * 128 + 128)
            interior = jt_lo <= jt < jt_hi
            kmat = kT if interior else (kTlo if jt in EXT_LO[qc] else kThi)
            ps = psS.tile([128, 1024], F32, name="psS", tag="ps")
            for h in range(2):
                hs = slice(h * 64, h * 64 + 64)
                nc.tensor.matmul(ps[:, h * 512:h * 512 + 512], kmat[hs, js],
                                 qT[hs, qs], start=True, stop=not interior,
                                 tile_position=(h * 64, 0))
            if interior:
                k = jt - jt_lo
                for h in range(2):
                    nc.tensor.matmul(ps[:, h * 512:h * 512 + 512], sident[:],
                                     skw[h][:, k, :], start=False, stop=True)
            et = etp.tile([128, 1024], BF16, name="expS", tag="expS")
            nc.scalar.activation(out=et[:], in_=ps[:], func=AF.Exp)
            ets[jt] = et
            if jt >= PV_DELAY:
                emit_pv(jt - PV_DELAY)
            if fillers:
                fillers.pop(0)()
            if len(fillers) > NJT - 1 - jt:
                fillers.pop(0)()
        for jt in range(NJT - PV_DELAY, NJT):
            emit_pv(jt)
        # denominators -> dram; unnormalized attnT rows
        for h in range(2):
            dn = small.tile([1, 512], F32, name="dn", tag="dn")
            nc.vector.tensor_copy(out=dn[:],
                                  in_=pos[64:65, h * 512:h * 512 + 512])
            nc.gpsimd.dma_start(out=den[h:h + 1, qc * 512:qc * 512 + 512],
                                in_=dn[:])
            hs = slice(h * 64, h * 64 + 64)
            nc.vector.tensor_copy(out=attnTu[hs, qs],
                                  in_=pos[0:64, h * 512:h * 512 + 512])

    # ---------- emission schedule ----------
    # head phase: q/k proj (above), P'(0), v proj, first skew read
    for f in list(emit_P_block(0)):
        f()
    emit_v()
    skw = emit_skew_read(0)

    for qc in range(NQC):
        fillers = []
        if qc + 1 < NQC:
            fillers.extend(emit_P_block(qc + 1))
        if qc > 0:
            for qt in range(4):
                fillers.append(lambda qc=qc - 1, qt=qt: emit_outproj_chunk(qc, qt))
        emit_attn_qc(qc, skw, fillers)
        # any fillers not consumed inside the jt loop
        for f in fillers:
            f()
        if qc + 1 < NQC:
            skw = emit_skew_read(qc + 1)
    # final outproj for qc=3
    for qt in range(4):
        emit_outproj_chunk(3, qt)
    ctx.close()


_NC_CACHE = [None]


def _get_nc():
    if _NC_CACHE[0] is None:
        _NC_CACHE[0] = build_kernel()
    return _NC_CACHE[0]


def make_in_maps(x, Wq, Wkv, Wo, bo, rel_emb):
    xT = [np.ascontiguousarray(x[b].T).astype(np.float32) for b in range(2)]
    cols = np.arange(W)
    idx = np.clip(1535 - cols, 0, 1024)
    relT = np.empty((128, W), np.float32)
    relT[0:64] = rel_emb[idx].T
    relT[64:128] = relT[0:64]
    relT = relT.astype(ml_dtypes.bfloat16)          # reversed rel table
    clampc = np.empty((128, 2), np.float32)
    clampc[0:64, 0] = rel_emb[0]
    clampc[64:128, 0] = rel_emb[0]
    clampc[0:64, 1] = rel_emb[1024]
    clampc[64:128, 1] = rel_emb[1024]
    ident = np.eye(128, dtype=np.float16)
    in_maps = []
    for c in range(8):
        b, hp = c // 4, c % 4
        cs = slice(hp * 128, hp * 128 + 128)
        in_maps.append({
            "xT": xT[b],
            "wq2": np.ascontiguousarray(Wq[:, cs] / 8.0).astype(np.float32),
            "wk2": np.ascontiguousarray(Wkv[:, :512][:, cs]).astype(np.float32),
            "wv2": np.ascontiguousarray(Wkv[:, 512:][:, cs]).astype(np.float32),
            "wo2": np.ascontiguousarray(Wo[cs, :]).astype(ml_dtypes.bfloat16),
            "relT": relT,
            "clampc": clampc,
            "ident": ident,
        })
    return in_maps


def run(x, Wq, Wkv, Wo, bo, rel_emb, trace=False, trace_cores=None):
    nc = _get_nc()
    in_maps = make_in_maps(x, Wq, Wkv, Wo, bo, rel_emb)
    res = run_bass_kernel_spmd(nc, in_maps, core_ids=list(range(8)),
                               trace=trace, trace_cores=trace_cores)
    out = np.zeros((2, N, D), np.float64)
    for c in range(8):
        b = c // 4
        r = res.results[c]
        dden = np.asarray(r["den"], np.float64)            # [2, N]
        p0 = np.asarray(r["out0"], np.float64)             # [N, D]
        p1 = np.asarray(r["out1"], np.float64)
        out[b] += p0 / dden[0][:, None] + p1 / dden[1][:, None]
    out += np.asarray(bo, np.float64)[None, None, :]
    return out.astype(np.float32), res


def kernel(x, Wq, Wkv, Wo, bo, rel_emb):
    out, _ = run(np.asarray(x), np.asarray(Wq), np.asarray(Wkv),
                 np.asarray(Wo), np.asarray(bo), np.asarray(rel_emb))
    return out
